# revision 1
# baseline (speedup 1.0000x reference)
"""ApplyPolicyMap kernel for Trainium2 (8 NeuronCores, pure data parallel).

Reference computes out[B,1858] = inputs.reshape(B,5120) @ pmap where pmap is a
0/1 one-hot selection matrix: each output column j copies exactly one input
column rows[j].  So the kernel is a column gather, executed on-device as an
indexed row gather over the batch-transposed shard:

  per core i (batch shard of 1024 rows, host-transposed to [5120, 1024]):
    gathered row j  = xt[rows[j], :]            (1858 contiguous 4 KiB reads)
    SBUF layout     = row j at partition j%128, free slot j//128
    DMA out         = [128, 15, 1024] f32 in DRAM (contiguous per partition)

Device implementations (KERNEL_IMPL env var; default dma_gather):
  dma_gather: Ant SWDGE dma_gather from the 'mlp' GPSIMD library, chunked
              (>= ~900 idxs in one call overflows the descriptor ring and
              hangs), spread over 4 SWDGE queues (parallel Q7 descriptor
              generation), per-chunk writeouts overlapped with later
              gathers.  DEFAULT: never crashed or mis-computed in any
              tested condition.  ~62-71us on HW (run-to-run variance of
              the shared chip is ~+/-5us; head is ~9us GPSIMD library
              load + 5us NEFF startup, data phase ~36us at the ~420 GB/s
              SBUF-fabric ceiling).
  indirect  : stock InstDMACopy indirect gather (no library load, ~57us
              best) — NOT shipped: HW honors only ONE index per partition
              per 2D-AP call, and spreading calls across SWDGE queues
              intermittently hard-crashes the device
              (NRT_EXEC_UNIT_UNRECOVERABLE); single-queue corrupts data
              when >4 DMAs are outstanding under NTFF profiling.
  hybrid    : indirect gathers during the library load, then dma_gather —
              crashes (library IRAM reload races in-flight stock SWDGE).

Host side: shard batch 8 ways, transpose each shard, derive the index form
of pmap (argmax over columns), and un-transpose the gathered result.
"""

import os

import numpy as np

C_IN = 5120
N_MOVES = 1858
B = 8192
NCORES = 8
BS = B // NCORES  # 1024 batch rows per core
NPAD = 1920  # N_MOVES rounded up to a multiple of 128
NSLOT = NPAD // 128  # 15
IDX_FREE = NPAD // 16  # 120
TAIL_P = N_MOVES - 128 * (NSLOT - 1)  # 66 valid partitions in the last slot

GATHER_CHUNK = 512  # dma_gather impl: idxs per call (multiple of 128, <=768)
NQUEUES = 4  # dma_gather impl: SWDGE queues
WSLOTS = 3  # indirect impl: slots per writeout group

IMPL = os.environ.get("KERNEL_IMPL") or "dma_gather"
if IMPL not in ("dma_gather", "indirect", "hybrid"):
    IMPL = "dma_gather"

# hybrid impl: slots 0..ISLOTS-1 via stock indirect DMA (runs during the
# GPSIMD library load), remaining slots via dma_gather on SWDGE queues 1-3.
ISLOTS = 4
HCHUNKS = [  # (j0, npad, nvalid, queue)
    (512, 512, 512, 1),
    (1024, 512, 512, 2),
    (1536, 384, N_MOVES - 1536, 3),
]

_cache = {}


def _build_indirect():
    """15 stock indirect row-gathers (128 rows each) spread round-robin over
    4 SWDGE queues (parallel Q7 descriptor generation, <=4 outstanding per
    ring), per-slot writeouts overlapping later gathers. No GPSIMD library."""
    import concourse.bacc as bacc
    import concourse.bass as bass
    import concourse.mybir as mybir

    nc = bacc.Bacc(num_swdge_queues=NQUEUES)

    xt = nc.declare_dram_parameter("xt", [C_IN, BS], mybir.dt.float32, isOutput=False)
    idx = nc.declare_dram_parameter(
        "idx", [128, NSLOT], mybir.dt.int32, isOutput=False
    )
    out = nc.declare_dram_parameter(
        "out", [128, NSLOT, BS], mybir.dt.float32, isOutput=True
    )

    with (
        nc.sbuf_tensor([128, NSLOT], mybir.dt.int32) as idx_sb,
        nc.sbuf_tensor([128, NSLOT, BS], mybir.dt.float32) as gbuf,
        nc.semaphore("hsem") as hsem,
        nc.semaphore("gsem0") as gsem0,
        nc.semaphore("gsem1") as gsem1,
        nc.semaphore("gsem2") as gsem2,
        nc.semaphore("gsem3") as gsem3,
        nc.Block() as block,
    ):
        gsems = [gsem0, gsem1, gsem2, gsem3]

        # Gather the high slots (11-14) first and write them out in one
        # combined DMA during the descriptor-generation-limited ramp; the
        # stream then ends on a single small writeout (slot 10), so the tail
        # pays only one ~2.5us sem receipt with little data left to move.
        order = [11, 12, 13, 14] + list(range(11))

        @block.sync
        def _(sync):
            sync.dma_start(idx_sb[:], idx[:]).then_inc(hsem, 16)
            n_wo = 1
            for q in range(NQUEUES):
                sync.wait_ge(gsems[q], 16)
            sync.dma_start(
                out[:, 11 : NSLOT - 1, :], gbuf[:, 11 : NSLOT - 1, :]
            ).then_inc(hsem, 16)
            sync.dma_start(
                out[:TAIL_P, NSLOT - 1, :], gbuf[:TAIL_P, NSLOT - 1, :]
            ).then_inc(hsem, 16)
            n_wo += 2
            for k, c in enumerate(order):
                if k < NQUEUES:
                    continue  # slots 11-14 handled above
                sync.wait_ge(gsems[k % NQUEUES], 16 * (k // NQUEUES + 1))
                sync.dma_start(out[:, c, :], gbuf[:, c, :]).then_inc(hsem, 16)
                n_wo += 1
            sync.wait_ge(hsem, 16 * n_wo)

        @block.gpsimd
        def _(g):
            g.wait_ge(hsem, 16)
            for k, c in enumerate(order):
                np_c = TAIL_P if c == NSLOT - 1 else 128  # skip pad rows
                inst = g.indirect_dma_start(
                    out=gbuf[:np_c, c, :],
                    out_offset=None,
                    in_=xt[:],
                    in_offset=bass.IndirectOffsetOnAxis(
                        ap=idx_sb[:np_c, c : c + 1], axis=0
                    ),
                )
                q = k % NQUEUES
                if q:
                    inst.ins.queue = f"qPoolDynamic{q}"
                inst.then_inc(gsems[q], 16)

    nc.compile()
    return nc


def _build_hybrid():
    import concourse.bacc as bacc
    import concourse.bass as bass
    import concourse.mybir as mybir
    from concourse import library_config

    nc = bacc.Bacc(num_swdge_queues=NQUEUES)

    xt = nc.declare_dram_parameter("xt", [C_IN, BS], mybir.dt.float32, isOutput=False)
    idx32 = nc.declare_dram_parameter(
        "idx32", [128, ISLOTS], mybir.dt.int32, isOutput=False
    )
    idx16 = nc.declare_dram_parameter(
        "idx16", [128, IDX_FREE], mybir.dt.int16, isOutput=False
    )
    out = nc.declare_dram_parameter(
        "out", [128, NSLOT, BS], mybir.dt.float32, isOutput=True
    )

    with (
        nc.sbuf_tensor([128, ISLOTS], mybir.dt.int32) as idx32_sb,
        nc.sbuf_tensor([128, IDX_FREE], mybir.dt.int16) as idx16_sb,
        nc.sbuf_tensor([128, NSLOT, BS], mybir.dt.float32) as gbuf,
        nc.semaphore("hsem") as hsem,
        nc.semaphore("isem") as isem,
        nc.semaphore("gsem1") as gsem1,
        nc.semaphore("gsem2") as gsem2,
        nc.semaphore("gsem3") as gsem3,
        nc.Block() as block,
    ):
        gsems = {1: gsem1, 2: gsem2, 3: gsem3}

        @block.sync
        def _(sync):
            sync.dma_start(idx32_sb[:], idx32[:]).then_inc(hsem, 16)
            sync.dma_start(idx16_sb[:], idx16[:]).then_inc(hsem, 16)
            n_wo = 2
            # indirect slots, written out as they land
            for c in range(ISLOTS):
                sync.wait_ge(isem, 16 * (c + 1))
                sync.dma_start(out[:, c, :], gbuf[:, c, :]).then_inc(hsem, 16)
                n_wo += 1
            # dma_gather chunks
            for j0, npad_c, nvalid_c, q in HCHUNKS:
                sync.wait_ge(gsems[q], 16)
                s0 = j0 // 128
                ns = npad_c // 128
                last = j0 + npad_c >= NPAD
                if last:
                    ns -= 1
                if ns > 0:
                    sync.dma_start(
                        out[:, s0 : s0 + ns, :], gbuf[:, s0 : s0 + ns, :]
                    ).then_inc(hsem, 16)
                    n_wo += 1
                if last:
                    sync.dma_start(
                        out[:TAIL_P, NSLOT - 1, :], gbuf[:TAIL_P, NSLOT - 1, :]
                    ).then_inc(hsem, 16)
                    n_wo += 1
            sync.wait_ge(hsem, 16 * n_wo)

        @block.gpsimd
        def _(g):
            g.wait_ge(hsem, 16)
            for c in range(ISLOTS):
                g.indirect_dma_start(
                    out=gbuf[:, c, :],
                    out_offset=None,
                    in_=xt[:],
                    in_offset=bass.IndirectOffsetOnAxis(
                        ap=idx32_sb[:, c : c + 1], axis=0
                    ),
                ).then_inc(isem, 16)
            g.load_library(library_config.mlp)
            g.wait_ge(hsem, 32)
            for j0, npad_c, nvalid_c, q in HCHUNKS:
                s0 = j0 // 128
                g.dma_gather(
                    gbuf[:, s0 : s0 + npad_c // 128, :],
                    xt[:],
                    idx16_sb[:, j0 // 16 : (j0 + npad_c) // 16],
                    npad_c,
                    nvalid_c,
                    BS,
                    queue_num=q,
                ).then_inc(gsems[q], 16)

    nc.compile()
    return nc


def _build_dma_gather():
    import concourse.bacc as bacc
    import concourse.mybir as mybir
    from concourse import library_config

    nc = bacc.Bacc(num_swdge_queues=NQUEUES)

    xt = nc.declare_dram_parameter("xt", [C_IN, BS], mybir.dt.float32, isOutput=False)
    idx = nc.declare_dram_parameter(
        "idx", [128, IDX_FREE], mybir.dt.int16, isOutput=False
    )
    out = nc.declare_dram_parameter(
        "out", [128, NSLOT, BS], mybir.dt.float32, isOutput=True
    )

    # Four even 512-idx chunks, one per SWDGE queue. Measured best of all
    # tested shapes (small-first openers and finer chunking both regress:
    # round-2 dispatches serialize on the Pool sequencer and writeout
    # boundaries drift off slot groups).
    chunks = []  # (j0, npad_chunk, nvalid_chunk)
    j = 0
    while j < NPAD:
        npad_c = min(GATHER_CHUNK, NPAD - j)
        chunks.append((j, npad_c, max(0, min(N_MOVES - j, npad_c))))
        j += npad_c

    with (
        nc.sbuf_tensor([128, IDX_FREE], mybir.dt.int16) as idx_sb,
        nc.sbuf_tensor([128, NSLOT, BS], mybir.dt.float32) as gbuf,
        nc.semaphore("hsem") as hsem,
        nc.semaphore("gsem0") as gsem0,
        nc.semaphore("gsem1") as gsem1,
        nc.semaphore("gsem2") as gsem2,
        nc.semaphore("gsem3") as gsem3,
        nc.Block() as block,
    ):
        gsems = [gsem0, gsem1, gsem2, gsem3]

        @block.sync
        def _(sync):
            sync.dma_start(idx_sb[:], idx[:]).then_inc(hsem, 16)
            n_wo = 0
            seen_per_queue = [0] * NQUEUES
            for c, (j0, npad_c, nvalid_c) in enumerate(chunks):
                q = c % NQUEUES
                seen_per_queue[q] += 1
                sync.wait_ge(gsems[q], 16 * seen_per_queue[q])
                s0 = j0 // 128
                ns = npad_c // 128
                last = j0 + npad_c >= NPAD
                if last:
                    ns -= 1  # final slot is partial (TAIL_P partitions)
                if ns > 0:
                    sync.dma_start(
                        out[:, s0 : s0 + ns, :], gbuf[:, s0 : s0 + ns, :]
                    ).then_inc(hsem, 16)
                    n_wo += 1
                if last:
                    sync.dma_start(
                        out[:TAIL_P, NSLOT - 1, :], gbuf[:TAIL_P, NSLOT - 1, :]
                    ).then_inc(hsem, 16)
                    n_wo += 1
            sync.wait_ge(hsem, 16 * (1 + n_wo))

        @block.gpsimd
        def _(g):
            g.load_library(library_config.mlp)
            g.wait_ge(hsem, 16)
            for c, (j0, npad_c, nvalid_c) in enumerate(chunks):
                q = c % NQUEUES
                s0 = j0 // 128
                g.dma_gather(
                    gbuf[:, s0 : s0 + npad_c // 128, :],
                    xt[:],
                    idx_sb[:, j0 // 16 : (j0 + npad_c) // 16],
                    npad_c,
                    nvalid_c,
                    BS,
                    queue_num=q,
                ).then_inc(gsems[q], 16)

    nc.compile()
    return nc


def _wrap_indices_i16(rows: np.ndarray) -> np.ndarray:
    """dma_gather form: int16 [128, IDX_FREE], idx j at (partition j%16,
    slot j//16), 16-row block replicated 8x (one replica per Q7 core)."""
    flat = np.full((NPAD,), -1, dtype=np.int16)
    flat[:N_MOVES] = rows.astype(np.int16)
    wrapped = flat.reshape(IDX_FREE, 16).T  # [16, IDX_FREE]
    return np.ascontiguousarray(np.tile(wrapped, (8, 1)))  # [128, IDX_FREE]


def _wrap_indices_i32(rows: np.ndarray) -> np.ndarray:
    """indirect form: int32 [128, NSLOT], idx[p, c] = rows_padded[c*128+p].
    Pad rows gather row 0; those slots are never written out."""
    flat = np.zeros((NPAD,), dtype=np.int32)
    flat[:N_MOVES] = rows.astype(np.int32)
    return np.ascontiguousarray(flat.reshape(NSLOT, 128).T)


def kernel(inputs: np.ndarray, pmap: np.ndarray) -> np.ndarray:
    from concourse.bass_utils import run_bass_kernel_spmd

    x = np.ascontiguousarray(np.asarray(inputs, dtype=np.float32)).reshape(B, C_IN)
    pm = np.asarray(pmap)
    rows = np.argmax(pm, axis=0)  # [1858] the one-hot row per output column

    if IMPL == "hybrid":
        idx_map = {
            "idx32": np.ascontiguousarray(_wrap_indices_i32(rows)[:, :ISLOTS]),
            "idx16": _wrap_indices_i16(rows),
        }
    elif IMPL == "indirect":
        idx_map = {"idx": _wrap_indices_i32(rows)}
    else:
        idx_map = {"idx": _wrap_indices_i16(rows)}

    in_maps = []
    for i in range(NCORES):
        shard = x[i * BS : (i + 1) * BS]  # [1024, 5120]
        xt = np.ascontiguousarray(shard.T)  # [5120, 1024]
        in_maps.append({"xt": xt, **idx_map})

    if "nc" not in _cache:
        builders = {
            "hybrid": _build_hybrid,
            "indirect": _build_indirect,
            "dma_gather": _build_dma_gather,
        }
        _cache["nc"] = builders[IMPL]()
    nc = _cache["nc"]

    trace = os.environ.get("KERNEL_TRACE", "") not in ("", "0")
    res = run_bass_kernel_spmd(nc, in_maps, list(range(NCORES)), trace=trace)
    if trace and res.exec_time_ns is not None:
        print(f"HW exec time: {res.exec_time_ns} ns")

    out = np.empty((B, N_MOVES), dtype=np.float32)
    for i in range(NCORES):
        o = np.asarray(res.results[i]["out"])  # [128, NSLOT, BS]
        ot = o.transpose(1, 0, 2).reshape(NPAD, BS)[:N_MOVES]  # [1858, 1024]
        out[i * BS : (i + 1) * BS, :] = ot.T
    return out



# revision 2
# speedup vs baseline: 1.9056x; 1.9056x over previous
"""ApplyPolicyMap kernel for Trainium2 (8 NeuronCores).

Reference computes out[B,1858] = inputs.reshape(B,5120) @ pmap where pmap is a
0/1 one-hot selection matrix: each output column j copies exactly one input
column rows[j].  So the kernel is a column gather.

Sharding (v2, "move-sharded"): split the 1858 MOVES across the 8 cores
(~233 each, sorted by source row), not the batch.  Each core gathers its
~233 rows from a contiguous 768-row band of the batch-transposed table
xt[5120, 8192] in bf16.  Rows are full batch width: 8192 * 2B = 16 KiB per
descriptor, 8x fewer and 4x bigger descriptors than the batch-sharded
baseline whose 4 KiB descriptors were engine-overhead-bound (~410 ns each,
~9.7 GB/s/engine).  Sorting the gather gives ascending HBM addresses.

bf16 (not fp16): max rel err is uniformly 2^-8 = 3.9e-3 over the whole
normal range (fp16 subnormals below 6e-5 would risk the 2e-2 gate near the
1e-6 denominator clamp).  Gate is rel_err < 2e-2; bf16 passes with 5x margin.

Device implementations (KERNEL_IMPL env var):
  indirect_ms : stock InstDMACopy indirect gather, single SWDGE queue,
                2 calls (128 + 105 idx; one index per partition per call),
                <=2 outstanding — the configuration prior experiments found
                safe (crashes only occurred with multi-queue spreading or
                >4 outstanding).  No GPSIMD library load (~8 us saved).
  gather_ms   : Ant SWDGE dma_gather ('mlp' library), 2 chunks on 2 queues.
                Proven-robust path, pays the ~8 us library load.
  dma_gather  : legacy batch-sharded f32 baseline (~68 us), emergency
                fallback.

Host side: derive rows = argmax(pmap), sort, split 8 ways, slice per-core
768-row bands (rebasing indices), convert to bf16; after the run, convert
back to f32 and un-permute columns.
"""

import os

import numpy as np

C_IN = 5120
N_MOVES = 1858
B = 8192
NCORES = 8
BS = B // NCORES  # legacy batch shard

# --- move-sharded (v2) constants ---
BAND = 768  # per-core row band (max span across cores is 718)
NSLOT_MS = 2  # gather calls per core: 128 + 105 indices
CALL_SIZES = (128, 105)  # sum 233 >= per-core move count (233 or 232)
NPAD_MS = 256  # idx tensor slots (2 columns of 128)
IDX16_FREE = NPAD_MS // 16  # 16

# --- legacy (batch-sharded) constants ---
NPAD = 1920
NSLOT = NPAD // 128  # 15
IDX_FREE = NPAD // 16  # 120
TAIL_P = N_MOVES - 128 * (NSLOT - 1)  # 66
GATHER_CHUNK = 512
NQUEUES = 4

IMPL = os.environ.get("KERNEL_IMPL") or "indirect_ms"
if IMPL not in ("indirect_ms", "gather_ms", "dma_gather"):
    IMPL = "indirect_ms"

_cache = {}


def _build_indirect_ms():
    """Move-sharded bf16 gather via stock indirect DMA on one SWDGE queue.

    2 calls x <=128 rows x 16 KiB, writeouts on the sync HWDGE ring overlap
    the second gather.  No GPSIMD library."""
    import concourse.bacc as bacc
    import concourse.bass as bass
    import concourse.mybir as mybir

    nc = bacc.Bacc(num_swdge_queues=1)

    xt = nc.declare_dram_parameter("xt", [BAND, B], mybir.dt.bfloat16, isOutput=False)
    idx = nc.declare_dram_parameter(
        "idx", [128, NSLOT_MS], mybir.dt.int32, isOutput=False
    )
    out = nc.declare_dram_parameter(
        "out", [128, NSLOT_MS, B], mybir.dt.bfloat16, isOutput=True
    )

    with (
        nc.sbuf_tensor([128, NSLOT_MS], mybir.dt.int32) as idx_sb,
        nc.sbuf_tensor([128, NSLOT_MS, B], mybir.dt.bfloat16) as gbuf,
        nc.semaphore("hsem") as hsem,
        nc.semaphore("isem") as isem,
        nc.Block() as block,
    ):

        @block.sync
        def _(sync):
            sync.dma_start(idx_sb[:], idx[:]).then_inc(hsem, 16)
            for c, np_c in enumerate(CALL_SIZES):
                sync.wait_ge(isem, 16 * (c + 1))
                sync.dma_start(out[:np_c, c, :], gbuf[:np_c, c, :]).then_inc(hsem, 16)
            sync.wait_ge(hsem, 16 * (1 + NSLOT_MS))

        @block.gpsimd
        def _(g):
            g.wait_ge(hsem, 16)
            for c, np_c in enumerate(CALL_SIZES):
                g.indirect_dma_start(
                    out=gbuf[:np_c, c, :],
                    out_offset=None,
                    in_=xt[:],
                    in_offset=bass.IndirectOffsetOnAxis(
                        ap=idx_sb[:np_c, c : c + 1], axis=0
                    ),
                ).then_inc(isem, 16)

    nc.compile()
    return nc


def _build_gather_ms():
    """Move-sharded bf16 gather via the 'mlp' GPSIMD dma_gather library,
    2 chunks of 128 idx on 2 SWDGE queues."""
    import concourse.bacc as bacc
    import concourse.mybir as mybir
    from concourse import library_config

    nc = bacc.Bacc(num_swdge_queues=2)

    xt = nc.declare_dram_parameter("xt", [BAND, B], mybir.dt.bfloat16, isOutput=False)
    idx = nc.declare_dram_parameter(
        "idx", [128, IDX16_FREE], mybir.dt.int16, isOutput=False
    )
    out = nc.declare_dram_parameter(
        "out", [128, NSLOT_MS, B], mybir.dt.bfloat16, isOutput=True
    )

    with (
        nc.sbuf_tensor([128, IDX16_FREE], mybir.dt.int16) as idx_sb,
        nc.sbuf_tensor([128, NSLOT_MS, B], mybir.dt.bfloat16) as gbuf,
        nc.semaphore("hsem") as hsem,
        nc.semaphore("gsem0") as gsem0,
        nc.semaphore("gsem1") as gsem1,
        nc.Block() as block,
    ):
        gsems = [gsem0, gsem1]

        @block.sync
        def _(sync):
            sync.dma_start(idx_sb[:], idx[:]).then_inc(hsem, 16)
            for c, np_c in enumerate(CALL_SIZES):
                sync.wait_ge(gsems[c], 16)
                sync.dma_start(out[:np_c, c, :], gbuf[:np_c, c, :]).then_inc(hsem, 16)
            sync.wait_ge(hsem, 16 * (1 + NSLOT_MS))

        @block.gpsimd
        def _(g):
            g.load_library(library_config.mlp)
            g.wait_ge(hsem, 16)
            for c, np_c in enumerate(CALL_SIZES):
                g.dma_gather(
                    gbuf[:, c : c + 1, :],
                    xt[:],
                    idx_sb[:, c * 8 : c * 8 + 8],
                    128,
                    np_c,
                    B,
                    queue_num=c,
                ).then_inc(gsems[c], 16)

    nc.compile()
    return nc


def _build_dma_gather():
    """Legacy batch-sharded f32 dma_gather baseline (see git history)."""
    import concourse.bacc as bacc
    import concourse.mybir as mybir
    from concourse import library_config

    nc = bacc.Bacc(num_swdge_queues=NQUEUES)

    xt = nc.declare_dram_parameter("xt", [C_IN, BS], mybir.dt.float32, isOutput=False)
    idx = nc.declare_dram_parameter(
        "idx", [128, IDX_FREE], mybir.dt.int16, isOutput=False
    )
    out = nc.declare_dram_parameter(
        "out", [128, NSLOT, BS], mybir.dt.float32, isOutput=True
    )

    chunks = []
    j = 0
    while j < NPAD:
        npad_c = min(GATHER_CHUNK, NPAD - j)
        chunks.append((j, npad_c, max(0, min(N_MOVES - j, npad_c))))
        j += npad_c

    with (
        nc.sbuf_tensor([128, IDX_FREE], mybir.dt.int16) as idx_sb,
        nc.sbuf_tensor([128, NSLOT, BS], mybir.dt.float32) as gbuf,
        nc.semaphore("hsem") as hsem,
        nc.semaphore("gsem0") as gsem0,
        nc.semaphore("gsem1") as gsem1,
        nc.semaphore("gsem2") as gsem2,
        nc.semaphore("gsem3") as gsem3,
        nc.Block() as block,
    ):
        gsems = [gsem0, gsem1, gsem2, gsem3]

        @block.sync
        def _(sync):
            sync.dma_start(idx_sb[:], idx[:]).then_inc(hsem, 16)
            n_wo = 0
            seen_per_queue = [0] * NQUEUES
            for c, (j0, npad_c, nvalid_c) in enumerate(chunks):
                q = c % NQUEUES
                seen_per_queue[q] += 1
                sync.wait_ge(gsems[q], 16 * seen_per_queue[q])
                s0 = j0 // 128
                ns = npad_c // 128
                last = j0 + npad_c >= NPAD
                if last:
                    ns -= 1
                if ns > 0:
                    sync.dma_start(
                        out[:, s0 : s0 + ns, :], gbuf[:, s0 : s0 + ns, :]
                    ).then_inc(hsem, 16)
                    n_wo += 1
                if last:
                    sync.dma_start(
                        out[:TAIL_P, NSLOT - 1, :], gbuf[:TAIL_P, NSLOT - 1, :]
                    ).then_inc(hsem, 16)
                    n_wo += 1
            sync.wait_ge(hsem, 16 * (1 + n_wo))

        @block.gpsimd
        def _(g):
            g.load_library(library_config.mlp)
            g.wait_ge(hsem, 16)
            for c, (j0, npad_c, nvalid_c) in enumerate(chunks):
                q = c % NQUEUES
                s0 = j0 // 128
                g.dma_gather(
                    gbuf[:, s0 : s0 + npad_c // 128, :],
                    xt[:],
                    idx_sb[:, j0 // 16 : (j0 + npad_c) // 16],
                    npad_c,
                    nvalid_c,
                    BS,
                    queue_num=q,
                ).then_inc(gsems[q], 16)

    nc.compile()
    return nc


def _wrap_indices_i16(flat: np.ndarray) -> np.ndarray:
    """dma_gather idx form: int16, idx j at (partition j%16, slot j//16),
    16-row block replicated 8x (one replica per Q7 core)."""
    n = len(flat)
    wrapped = flat.astype(np.int16).reshape(n // 16, 16).T  # [16, n//16]
    return np.ascontiguousarray(np.tile(wrapped, (8, 1)))  # [128, n//16]


def _move_shard_plan(pm: np.ndarray):
    """Split moves across cores sorted by source row; per-core band + idx."""
    rows = np.argmax(pm, axis=0)  # [1858] one-hot row per output column
    order = np.argsort(rows, kind="stable")
    parts = np.array_split(order, NCORES)  # move ids per core, row-sorted
    plan = []
    for part in parts:
        r = rows[part]  # sorted ascending
        start = int(min(r[0], C_IN - BAND))
        rebased = (r - start).astype(np.int64)
        assert rebased.min() >= 0 and rebased.max() < BAND
        flat = np.zeros(NPAD_MS, dtype=np.int64)
        flat[: len(rebased)] = rebased
        plan.append((part, start, flat, len(rebased)))
    return plan


def _kernel_move_sharded(x: np.ndarray, pm: np.ndarray, trace: bool) -> np.ndarray:
    import ml_dtypes
    from concourse.bass_utils import run_bass_kernel_spmd

    bf16 = ml_dtypes.bfloat16
    xt = np.ascontiguousarray(x.reshape(B, C_IN).T).astype(bf16)  # [5120, 8192]

    plan = _move_shard_plan(pm)
    in_maps = []
    for part, start, flat, nval in plan:
        band = np.ascontiguousarray(xt[start : start + BAND])  # [768, 8192] bf16
        if IMPL == "gather_ms":
            f = flat.copy()
            f[nval:] = -1  # dma_gather skips trailing negatives
            # nvalid is passed per call; pad inside call 1's 128 block -> -1
            idx_map = _wrap_indices_i16(f)
        else:
            idx_map = np.ascontiguousarray(
                flat.reshape(NSLOT_MS, 128).T.astype(np.int32)
            )  # [128, 2]
        in_maps.append({"xt": band, "idx": idx_map})

    if "nc" not in _cache:
        _cache["nc"] = (
            _build_gather_ms() if IMPL == "gather_ms" else _build_indirect_ms()
        )
    nc = _cache["nc"]

    res = run_bass_kernel_spmd(nc, in_maps, list(range(NCORES)), trace=trace)
    if trace and res.exec_time_ns is not None:
        print(f"HW exec time: {res.exec_time_ns} ns")

    out = np.empty((B, N_MOVES), dtype=np.float32)
    for i, (part, start, flat, nval) in enumerate(plan):
        o = np.asarray(res.results[i]["out"])  # [128, 2, 8192] bf16
        rows_g = o.transpose(1, 0, 2).reshape(NPAD_MS, B)[:nval]  # [nval, 8192]
        out[:, part] = rows_g.T.astype(np.float32)
    return out


def _kernel_legacy(x: np.ndarray, pm: np.ndarray, trace: bool) -> np.ndarray:
    from concourse.bass_utils import run_bass_kernel_spmd

    rows = np.argmax(pm, axis=0)
    flat = np.full((NPAD,), -1, dtype=np.int64)
    flat[:N_MOVES] = rows
    idx_map = {"idx": _wrap_indices_i16(flat)}

    xf = x.reshape(B, C_IN)
    in_maps = []
    for i in range(NCORES):
        shard = xf[i * BS : (i + 1) * BS]
        in_maps.append({"xt": np.ascontiguousarray(shard.T), **idx_map})

    if "nc" not in _cache:
        _cache["nc"] = _build_dma_gather()
    nc = _cache["nc"]

    res = run_bass_kernel_spmd(nc, in_maps, list(range(NCORES)), trace=trace)
    if trace and res.exec_time_ns is not None:
        print(f"HW exec time: {res.exec_time_ns} ns")

    out = np.empty((B, N_MOVES), dtype=np.float32)
    for i in range(NCORES):
        o = np.asarray(res.results[i]["out"])  # [128, NSLOT, BS]
        ot = o.transpose(1, 0, 2).reshape(NPAD, BS)[:N_MOVES]
        out[i * BS : (i + 1) * BS, :] = ot.T
    return out


def kernel(inputs: np.ndarray, pmap: np.ndarray) -> np.ndarray:
    x = np.ascontiguousarray(np.asarray(inputs, dtype=np.float32))
    pm = np.asarray(pmap)
    trace = os.environ.get("KERNEL_TRACE", "") not in ("", "0")
    if IMPL == "dma_gather":
        return _kernel_legacy(x, pm, trace)
    return _kernel_move_sharded(x, pm, trace)
